# revision 24
# baseline (speedup 1.0000x reference)
"""TRN2 Bass kernel for nn_MultiHeadAttention (GQA + RoPE + causal, dense transformer).

Sharding: tensor-parallel over kv-head groups (TP=4; each core owns 2 kv heads
plus their 8 grouped q heads) x data-parallel over batch (DP=2) -> 8 cores.
The host sums the 4 partial o-projection outputs per batch element (the TP
all-reduce) and transposes back.

Schedule (v2): single merged QKV projection pass over resident weights; causal
attention with u-pair-merged score tiles ([128,1024] f32 PSUM spanning two
banks) so each softmax exp is one wide ACT instruction; the attention-mask is
folded into V's appended ones-column, so exp needs no bias; normalization is
done inline in SBUF (denominator row copy -> reciprocal_approx_fast ->
gpsimd partition_broadcast -> one DVE multiply into resident attnT) with no
DRAM staging; the o-projection is emitted as filler between attention steps to
keep the PE dense (avoids HAM re-throttling to half clock).
"""
from collections import deque
from contextlib import ExitStack

import numpy as np
import ml_dtypes

import concourse.bass as bass
import concourse.mybir as mybir
import concourse.tile as tile
from concourse import bacc
from concourse.bass_utils import run_bass_kernel_spmd
from concourse.masks import make_identity

F32 = mybir.dt.float32
BF16 = mybir.dt.bfloat16
DT = BF16
AF = mybir.ActivationFunctionType

N_CORES = 8
B, S, D = 2, 2048, 2048
HQ_TOT, HKV_TOT, HD = 32, 8, 64
ROPE_BASE = 10000.0
TP = N_CORES // B          # 4 cores per batch element
HQ = HQ_TOT // TP          # 8 q heads per core
HKV = HKV_TOT // TP        # 2 kv heads per core

QC = HQ * HD               # 512 q proj cols
KC = 2 * HKV * HD          # 256 k+v proj cols
DK = D // 128              # 16 contraction k-tiles
T = S // 512               # 4 q-tiles
KT = S // 128              # 16 key blocks
QM = QC // 128             # 4 q m-tiles
NREP = HQ // HKV           # 4
LAG = 2                    # PV trails scores/exp by this many pair-steps


def build(nc):
    hsT = nc.dram_tensor("hsT", [D, S], DT, kind="ExternalInput")
    wq = nc.dram_tensor("wq", [D, QC], DT, kind="ExternalInput")
    wkv = nc.dram_tensor("wkv", [D, KC], DT, kind="ExternalInput")
    wo = nc.dram_tensor("wo", [QC, D], DT, kind="ExternalInput")
    cos4_d = nc.dram_tensor("cos4", [128, S], DT, kind="ExternalInput")
    sinpm_d = nc.dram_tensor("sinpm", [128, S], DT, kind="ExternalInput")
    maskmul_d = nc.dram_tensor("maskmul", [128, KT], F32, kind="ExternalInput")
    outT = nc.dram_tensor("outT", [D, S], DT, kind="ExternalOutput")

    ctx = ExitStack()
    with tile.TileContext(nc) as tc:
        consts = ctx.enter_context(tc.tile_pool(name="consts", bufs=1))
        big = ctx.enter_context(tc.tile_pool(name="big", bufs=1))
        hspool = ctx.enter_context(tc.tile_pool(name="hspool", bufs=10))
        hsrpool = ctx.enter_context(tc.tile_pool(name="hsrpool", bufs=2))
        prpool = ctx.enter_context(tc.tile_pool(name="prpool", bufs=6))
        npool = ctx.enter_context(tc.tile_pool(name="npool", bufs=3))
        opool = ctx.enter_context(tc.tile_pool(name="opool", bufs=4))
        pspool = ctx.enter_context(tc.tile_pool(name="pspool", bufs=1, space="PSUM"))

        # ---- constants / resident weights ----
        # weight-tile DMAs alternate between the scalar and sync queues so the
        # projection k-loop is never gated on a single queue's issue rate
        wkv_t, wq_t = [], []
        for k in range(DK):
            eng = nc.scalar if k % 2 == 0 else nc.sync
            wt = consts.tile([128, KC], DT, name=f"wkv{k}")
            eng.dma_start(wt[:], wkv[k * 128:(k + 1) * 128, :])
            wkv_t.append(wt)
            wt = consts.tile([128, QC], DT, name=f"wq{k}")
            eng.dma_start(wt[:], wq[k * 128:(k + 1) * 128, :])
            wq_t.append(wt)
        cos4 = consts.tile([128, S], DT)
        nc.scalar.dma_start(cos4[:], cos4_d[:])
        sinpm = consts.tile([128, S], DT)
        nc.scalar.dma_start(sinpm[:], sinpm_d[:])
        maskmul = consts.tile([128, KT], F32)
        nc.sync.dma_start(maskmul[:], maskmul_d[:])
        ident = consts.tile([128, 128], DT)
        make_identity(nc, ident[:])
        wo_res = [consts.tile([128, D], DT, name=f"wores{k}")
                  for k in range(QC // 128)]

        # ---- resident tensors ----
        kX = big.tile([128, S], DT, name="kX")            # k proj, rope'd in place
        vT = big.tile([128, S], DT, name="vT")            # v proj (dims x keys)
        qX = [big.tile([128, S], DT, name=f"qX{m}") for m in range(QM)]
        kR = [big.tile([128, S], DT, name=f"kR{j}") for j in range(HKV)]
        vext = [big.tile([128, 2 * 65], DT, name=f"vext{u}") for u in range(KT)]
        attnT = [big.tile([128, S], DT, name=f"attnT{k}") for k in range(QM)]

        def rope_chunk(X, ch):
            # in-place rope on X[:, 512-chunk]; partner 32-blocks via DMA copy
            sl = slice(ch * 512, (ch + 1) * 512)
            P = npool.tile([128, 512], DT, tag="ropeP", bufs=2)
            for blk in range(4):
                psrc = (blk ^ 1) * 32
                nc.sync.dma_start(P[blk * 32:blk * 32 + 32, :], X[psrc:psrc + 32, sl])
            m1 = npool.tile([128, 512], DT, tag="ropem1", bufs=2)
            nc.vector.tensor_mul(m1[:], X[:, sl], cos4[:, sl])
            m2 = npool.tile([128, 512], DT, tag="ropem2", bufs=2)
            nc.vector.tensor_mul(m2[:], P[:], sinpm[:, sl])
            nc.vector.tensor_add(X[:, sl], m1[:], m2[:])

        # ---- shared helpers ----
        def vext_build(ch):
            flv = pspool.tile([128, 512], DT, tag="fl", bufs=1, name=f"vt{ch}")
            for ul in range(4):
                u = 4 * ch + ul
                nc.tensor.transpose(flv[:, ul * 128:(ul + 1) * 128],
                                    vT[:, u * 128:(u + 1) * 128], ident[:])
            for ul in range(4):
                u = 4 * ch + ul
                dst = vext[u][:].rearrange("p (j cc) -> p j cc", j=2)[:, :, 0:64]
                vsrc = flv[:, ul * 128:(ul + 1) * 128].rearrange(
                    "p (j cc) -> p j cc", j=2)
                nc.vector.tensor_scalar_mul(dst, vsrc, maskmul[:, u:u + 1])
                nc.vector.tensor_copy(
                    vext[u][:].rearrange("p (j cc) -> p j cc", j=2)[:, :, 64:65],
                    maskmul[:, u:u + 1].rearrange("p (j cc) -> p j cc", j=1)
                    .broadcast_to((128, 2, 1)))

        def k_finish(ch):
            csl = slice(ch * 512, (ch + 1) * 512)
            rope_chunk(kX, ch)
            for j in range(HKV):
                for half in range(2):
                    nc.sync.dma_start(kR[j][half * 64:half * 64 + 64, csl],
                                      kX[j * 64:j * 64 + 64, csl])

        # ---- phase 1: merged kv+q projections for chunk 0 only ----
        for ch in range(1):
            csl = slice(ch * 512, (ch + 1) * 512)
            sc0 = pspool.tile([128, 1024], F32, tag="sc", bufs=2, name=f"pk{ch}")
            sc1 = pspool.tile([128, 1024], F32, tag="sc", bufs=2, name=f"pq{ch}")
            pa0 = pspool.tile([128, 512], F32, tag="pa", bufs=3, name=f"pq2{ch}")
            pa1 = pspool.tile([128, 512], F32, tag="pa", bufs=3, name=f"pq3{ch}")
            for k in range(DK):
                hs = hspool.tile([128, 512], DT, tag="hs", name=f"hs{ch}_{k}")
                nc.gpsimd.dma_start(hs[:], hsT[k * 128:(k + 1) * 128, csl])
                st, sp = k == 0, k == DK - 1
                nc.tensor.matmul(sc0[:, 0:512], wkv_t[k][:, 0:128], hs[:], start=st, stop=sp)
                nc.tensor.matmul(sc0[:, 512:1024], wkv_t[k][:, 128:256], hs[:], start=st, stop=sp)
                nc.tensor.matmul(sc1[:, 0:512], wq_t[k][:, 0:128], hs[:], start=st, stop=sp)
                nc.tensor.matmul(sc1[:, 512:1024], wq_t[k][:, 128:256], hs[:], start=st, stop=sp)
                nc.tensor.matmul(pa0[:], wq_t[k][:, 256:384], hs[:], start=st, stop=sp)
                nc.tensor.matmul(pa1[:], wq_t[k][:, 384:512], hs[:], start=st, stop=sp)
            nc.scalar.copy(kX[:, csl], sc0[:, 0:512])
            nc.scalar.copy(vT[:, csl], sc0[:, 512:1024])
            nc.scalar.copy(qX[0][:, csl], sc1[:, 0:512])
            nc.scalar.copy(qX[1][:, csl], sc1[:, 512:1024])
            nc.scalar.copy(qX[2][:, csl], pa0[:])
            nc.scalar.copy(qX[3][:, csl], pa1[:])
            k_finish(ch)
            for m in range(QM):
                rope_chunk(qX[m], ch)
            vext_build(ch)

        # o-proj weights load late: the early phase is DMA-bandwidth bound and
        # wo is first needed ~150us in (first o-proj filler)
        for k in range(QC // 128):
            nc.sync.dma_start(wo_res[k][:], wo[k * 128:(k + 1) * 128, :])

        # ---- phase 2: attention; proj chunks 2-3 and o-proj run as filler ----
        def proj_filler(ch):
            csl = slice(ch * 512, (ch + 1) * 512)
            hsr = hsrpool.tile([128, DK * 512], DT, tag="hsr", name=f"hsr{ch}")
            for k in range(DK):
                nc.gpsimd.dma_start(hsr[:, k * 512:(k + 1) * 512],
                                    hsT[k * 128:(k + 1) * 128, csl])
            yield
            specs = [("kv", 0, kX), ("kv", 1, vT)] + \
                    [("q", m, qX[m]) for m in range(QM)]
            for kind, m, dst in specs:
                fl = pspool.tile([128, 512], F32, tag="fl", bufs=1,
                                 name=f"pf{ch}_{kind}{m}")
                wt = wkv_t if kind == "kv" else wq_t
                for k in range(DK):
                    nc.tensor.matmul(fl[:], wt[k][:, m * 128:(m + 1) * 128],
                                     hsr[:, k * 512:(k + 1) * 512],
                                     start=(k == 0), stop=(k == DK - 1))
                    if k % 4 == 3 and k != DK - 1:
                        yield
                nc.vector.tensor_copy(dst[:, csl], fl[:])
                yield
                if kind == "kv" and m == 1:
                    k_finish(ch)
                    yield
                    vext_build(ch)
                    yield
                if kind == "q":
                    rope_chunk(dst, ch)
                    yield

        def oproj_filler(t):
            tsl = slice(t * 512, (t + 1) * 512)
            for mD in range(D // 128):
                if t == T - 1:
                    tag, bufs = ("pa", 3) if mD % 4 != 3 else ("fl", 1)
                else:
                    tag, bufs = "fl", 1
                fl = pspool.tile([128, 512], F32, tag=tag, bufs=bufs,
                                 name=f"po{t}_{mD}")
                for k in range(QM):
                    nc.tensor.matmul(fl[:], wo_res[k][:, mD * 128:(mD + 1) * 128],
                                     attnT[k][:, tsl], start=(k == 0),
                                     stop=(k == QM - 1))
                osb = opool.tile([128, 512], DT, tag="osb", name=f"ob{t}_{mD}")
                if t == T - 1 and mD % 2 == 0:
                    nc.scalar.copy(osb[:], fl[:])
                else:
                    nc.vector.tensor_copy(osb[:], fl[:])
                nc.sync.dma_start(outT[mD * 128:(mD + 1) * 128, tsl], osb[:])
                yield

        fillgens = deque()          # entries: [need_t, generator]
        fillgens.append([1, proj_filler(1)])
        fillgens.append([2, proj_filler(2)])
        fillgens.append([3, proj_filler(3)])

        def pump(n):
            while n > 0 and fillgens:
                try:
                    next(fillgens[0][1])
                    n -= 1
                except StopIteration:
                    fillgens.popleft()

        def drain_for(t):
            while fillgens and fillgens[0][0] <= t:
                try:
                    next(fillgens[0][1])
                except StopIteration:
                    fillgens.popleft()

        accums = {}
        pendingPV = deque()

        def emit_scores(hp, t, js):
            # one pair-step: heads (2hp, 2hp+1), key blocks u0=2js, u1=2js+1
            u0 = 2 * js
            j = hp // 2
            prs = []
            scs = []
            for hh in range(2):
                scs.append(pspool.tile([128, 1024], F32, tag="sc", bufs=2,
                                       name=f"s{hp}_{t}_{js}_{hh}"))
            for ul in range(2):
                u = u0 + ul
                w = u - 4 * t
                c0 = 128 * w if w > 0 else 0
                for hh in range(2):
                    qb = hh * 64
                    nc.tensor.matmul(
                        scs[hh][:, ul * 512 + c0:(ul + 1) * 512],
                        kR[j][qb:qb + 64, u * 128:(u + 1) * 128],
                        qX[hp][qb:qb + 64, t * 512 + c0:(t + 1) * 512],
                        start=True, stop=True)
            for hh in range(2):
                pr = prpool.tile([128, 1024], DT, tag="pr", name=f"pr{hp}_{t}_{js}_{hh}")
                nc.scalar.activation(pr[:], scs[hh][:], AF.Exp, scale=float(HD) ** -0.5)
                prs.append(pr)
            for ul in range(2):
                u = u0 + ul
                w = u - 4 * t
                if w < 0:
                    continue
                c0 = 128 * w
                for hh in range(2):
                    if c0 > 0:
                        nc.gpsimd.memset(prs[hh][:, ul * 512:ul * 512 + c0], 0.0)
                    nc.gpsimd.affine_select(
                        out=prs[hh][:, ul * 512 + c0:ul * 512 + c0 + 128],
                        in_=prs[hh][:, ul * 512 + c0:ul * 512 + c0 + 128],
                        compare_op=mybir.AluOpType.is_ge, fill=0.0,
                        base=0, channel_multiplier=-1, pattern=[[1, 128]])
            return (hp, t, js, prs)

        def emit_pv(item):
            hp, t, js, prs = item
            u0 = 2 * js
            j = hp // 2
            last = (u0 + 1 == 4 * t + 3)
            for hh in range(2):
                key = (hp, t, hh)
                if key not in accums:
                    accums[key] = pspool.tile([128, 512], F32, tag="pa", bufs=3,
                                              name=f"pa{2 * hp + hh}_{t}")
            for ul in range(2):
                u = u0 + ul
                for hh in range(2):
                    nc.tensor.matmul(accums[(hp, t, hh)][0:65, :],
                                     vext[u][:, j * 65:(j + 1) * 65],
                                     prs[hh][:, ul * 512:(ul + 1) * 512],
                                     start=(u == 0), stop=(last and ul == 1))
            for hh in range(2):
                h = 2 * hp + hh
                pa = accums[(hp, t, hh)]
                if last:
                    del accums[(hp, t, hh)]
                    tmpden = npool.tile([1, 512], F32, tag="td", name=f"td{h}_{t}")
                    nc.vector.tensor_copy(tmpden[:], pa[64:65, :])
                    rrow = npool.tile([1, 512], F32, tag="rr", name=f"rr{h}_{t}")
                    nc.vector.reciprocal_approx_fast(rrow[:], tmpden[:])
                    rbc = npool.tile([64, 512], F32, tag="rbc", name=f"rb{h}_{t}")
                    nc.gpsimd.partition_broadcast(rbc[:], rrow[:])
                    kk = h // 2
                    rb = (h % 2) * 64
                    nc.vector.tensor_mul(attnT[kk][rb:rb + 64, t * 512:(t + 1) * 512],
                                         pa[0:64, :], rbc[:])

        for t in range(T):
            drain_for(t)
            for hp in range(HQ // 2):
                for js in range(2 * t + 2):
                    pendingPV.append(emit_scores(hp, t, js))
                    pump(2)
                    if len(pendingPV) > LAG:
                        emit_pv(pendingPV.popleft())
            while pendingPV:
                emit_pv(pendingPV.popleft())
            fillgens.append([99, oproj_filler(t)])
        while fillgens:
            try:
                next(fillgens[0][1])
            except StopIteration:
                fillgens.popleft()
        ctx.close()
    return nc


def _host_prep(hidden_states, attention_mask, Wq, Wk, Wv, Wo):
    bf16 = ml_dtypes.bfloat16
    hs = np.asarray(hidden_states, np.float32)
    am = np.asarray(attention_mask)
    Wq = np.asarray(Wq, np.float32)
    Wk = np.asarray(Wk, np.float32)
    Wv = np.asarray(Wv, np.float32)
    Wo = np.asarray(Wo, np.float32)

    inv = 1.0 / (ROPE_BASE ** (np.arange(0, HD, 2, dtype=np.float64) / HD))
    freqs = np.arange(S, dtype=np.float64)[:, None] * inv[None, :]
    cosT = np.cos(freqs).T.astype(np.float32)
    sinT = np.sin(freqs).T.astype(np.float32)
    cos4 = np.tile(cosT, (4, 1))
    sign = np.repeat(np.array([-1.0, 1.0, -1.0, 1.0], np.float32), 32)[:, None]
    sinpm = np.tile(sinT, (4, 1)) * sign

    def perm_eo(wcols):  # head dims -> [evens | odds]
        return np.concatenate([wcols[:, 0::2], wcols[:, 1::2]], axis=1)

    hsT_b = [np.ascontiguousarray(hs[b].T).astype(bf16) for b in range(B)]
    mm_b = []
    for b in range(B):
        mm = (am[b] > 0).astype(np.float32)
        mm_b.append(np.ascontiguousarray(mm.reshape(S // 128, 128).T))

    in_maps = []
    for core in range(N_CORES):
        b, g = core // TP, core % TP
        heads = range(g * HQ, (g + 1) * HQ)
        kvs = range(g * HKV, (g + 1) * HKV)
        wq_c = np.concatenate([perm_eo(Wq[:, h * HD:(h + 1) * HD]) for h in heads], 1)
        wk_c = np.concatenate([perm_eo(Wk[:, j * HD:(j + 1) * HD]) for j in kvs], 1)
        wv_c = np.concatenate([Wv[:, j * HD:(j + 1) * HD] for j in kvs], 1)
        wkv_c = np.ascontiguousarray(np.concatenate([wk_c, wv_c], 1))
        wo_c = np.ascontiguousarray(
            np.concatenate([Wo[h * HD:(h + 1) * HD, :] for h in heads], 0))
        in_maps.append({
            "hsT": hsT_b[b],
            "wq": np.ascontiguousarray(wq_c).astype(bf16),
            "wkv": wkv_c.astype(bf16),
            "wo": wo_c.astype(bf16),
            "cos4": cos4.astype(bf16),
            "sinpm": sinpm.astype(bf16),
            "maskmul": mm_b[b],
        })
    return in_maps


_NC_CACHE = {}


def _get_nc():
    if "nc" not in _NC_CACHE:
        nc = bacc.Bacc("TRN2", target_bir_lowering=False, num_devices=N_CORES)
        build(nc)
        nc.compile()
        _NC_CACHE["nc"] = nc
    return _NC_CACHE["nc"]


def kernel(hidden_states, attention_mask, Wq, Wk, Wv, Wo):
    nc = _get_nc()
    in_maps = _host_prep(hidden_states, attention_mask, Wq, Wk, Wv, Wo)
    res = run_bass_kernel_spmd(nc, in_maps, list(range(N_CORES)))
    out = np.zeros((B, S, D), np.float32)
    for core, r in enumerate(res.results):
        out[core // TP] += np.asarray(r["outT"], dtype=np.float32).T
    return out
